# revision 5
# baseline (speedup 1.0000x reference)
"""DecayAttention Trainium2 kernel — 8-core SPMD.

Problem: B=2, L=2048, D=1024, H=16 heads (Hd=64).
  out = (softmax(Q K^T/sqrt(Hd) - rate_h*log1p(|i-j|) + causal) V) @ Wo.T + bo

Sharding: core c handles batch b = c//4 and heads h in [4*(c%4), 4*(c%4)+4).
Q/K/V projections column-sharded, Wo row-sharded; the 4 cores of each batch
return partial outputs that the host sums (plus Wo@bv + bo, both of which are
q-independent constants because softmax rows sum to 1).

Device-side layout tricks:
  - x is shipped pre-transposed (xT [D, L]) so every matmul contraction dim
    sits on partitions; no on-device transposes at all.
  - Q^T/K^T [64, L] per head come straight out of the projection matmuls.
  - scores are computed transposed (S^T[k, q] = K Q^T) so softmax's k-reduction
    becomes a matmul contraction: V is augmented with a ones column and
    P^T = exp(S^T) * expA gives numerator and denominator in one PV matmul.
  - decay bias + causal mask collapse into one Toeplitz factor
    expA[k, q] = (1+|q-k|)^(-rate) * [k <= q], materialized per tile by a
    single DMA from a 4095-float vector with a diagonal access pattern
    [[1, 128], [-1, 512]].
  - matmul operands use float32r (TF32, full PE rate at N>=256).
"""
import math

import numpy as np

import concourse.bass as bass
import concourse.mybir as mybir
import concourse.tile as tile
from concourse import bass_utils

f32 = mybir.dt.float32
f32r = mybir.dt.float32r
Exp = mybir.ActivationFunctionType.Exp

B, L, D, H = 2, 2048, 1024, 16
Hd = D // H                      # 64
N_CORES = 8
CPB = N_CORES // B               # 4 cores per batch element
HPC = H // CPB                   # 4 heads per core
DHC = HPC * Hd                   # 256 head-dims per core
NQ = L // 512                    # 4 q-chunks of 512
NLT = L // 128                   # 16 l/k tiles of 128
NE = D // 128                    # 8 contraction tiles for projections
GLEN = 2 * L - 1                 # 4095
GOFF = L - 1                     # 2047


def _split_multi_waits(nc):
    """This container's walrus accepts at most one sync-wait per engine
    instruction; hoist extras onto single-wait NOPs placed just before."""
    for fn in nc.m.functions:
        for blk in fn.blocks:
            out, changed = [], False
            for inst in blk.instructions:
                si = inst.sync_info
                waits = list(si.on_wait) if si is not None and si.on_wait else []
                if len(waits) > 1:
                    changed = True
                    for w in waits[:-1]:
                        nop = mybir.InstNoOp(
                            name=nc.get_next_instruction_name(), ins=[], outs=[])
                        nop.engine = inst.engine
                        nop.sync_info = mybir.SyncInfo(on_wait=[w], on_update=[])
                        out.append(nop)
                    inst.sync_info = mybir.SyncInfo(
                        on_wait=[waits[-1]], on_update=list(si.on_update or []))
                out.append(inst)
            if changed:
                blk.instructions = out


def build_nc(n_g: int):
    """Build the per-core Bass program. n_g = 1 (all heads share one decay
    rate, the setup_inputs case) or HPC (per-head expA vectors)."""
    nc = bass.Bass("TRN2", target_bir_lowering=False, debug=False)

    xT = nc.dram_tensor("xT", [D, L], f32r, kind="ExternalInput").ap()
    wqT = nc.dram_tensor("wqT", [D, DHC], f32r, kind="ExternalInput").ap()
    wkT = nc.dram_tensor("wkT", [D, DHC], f32r, kind="ExternalInput").ap()
    wvT = nc.dram_tensor("wvT", [D, DHC], f32r, kind="ExternalInput").ap()
    woT = nc.dram_tensor("woT", [DHC, D], f32r, kind="ExternalInput").ap()
    bq = nc.dram_tensor("bq", [DHC, 1], f32, kind="ExternalInput").ap()
    bk = nc.dram_tensor("bk", [DHC, 1], f32, kind="ExternalInput").ap()
    g = nc.dram_tensor("g", [n_g, GLEN], f32, kind="ExternalInput")
    pmask = nc.dram_tensor("pmask", [L, 1], f32, kind="ExternalInput").ap()
    out = nc.dram_tensor("out", [L, D], f32, kind="ExternalOutput").ap()

    with tile.TileContext(nc) as tc:
        with tc.tile_pool(name="cons", bufs=1) as cons:
            # persistent SBUF residents
            qt_h = [cons.tile([Hd, L], f32r, name=f"qt{h}") for h in range(HPC)]
            kt_h = [cons.tile([Hd, L], f32r, name=f"kt{h}") for h in range(HPC)]
            vaug = [cons.tile([128, 65 * HPC], f32r, name=f"vaug{t}")
                    for t in range(NLT)]
            woT_h = [cons.tile([Hd, D], f32r, name=f"wo{h}") for h in range(HPC)]
            bq_h = [cons.tile([Hd, 1], f32, name=f"bq{h}") for h in range(HPC)]
            bk_h = [cons.tile([Hd, 1], f32, name=f"bk{h}") for h in range(HPC)]
            ones64 = cons.tile([1, Hd], f32r)
            ones_st = cons.tile([128, HPC], f32)
            outT_h = [cons.tile([Hd, 512], f32r, name=f"ot{h}") for h in range(HPC)]

            ones_st64 = cons.tile([1, Hd], f32)
            nc.vector.memset(ones_st[:, :], 1.0)
            nc.vector.memset(ones_st64[:, :], 1.0)
            nc.vector.tensor_copy(ones64[:, :], ones_st64[:, :])
            for h in range(HPC):
                nc.sync.dma_start(woT_h[h][:, :], woT[h * Hd:(h + 1) * Hd, :])
                nc.sync.dma_start(bq_h[h][:, :], bq[h * Hd:(h + 1) * Hd, :])
                nc.sync.dma_start(bk_h[h][:, :], bk[h * Hd:(h + 1) * Hd, :])

            # ---- Phase A: projections ----
            with tc.tile_pool(name="xw", bufs=1) as xw, \
                 tc.tile_pool(name="psA", bufs=3, space="PSUM") as psA, \
                 tc.tile_pool(name="wkA", bufs=3) as wkA:
                xt_t = [xw.tile([128, L], f32r, name=f"x{e}") for e in range(NE)]
                wq_t = [xw.tile([128, DHC], f32r, name=f"wq{e}") for e in range(NE)]
                wk_t = [xw.tile([128, DHC], f32r, name=f"wk{e}") for e in range(NE)]
                wv_t = [xw.tile([128, DHC], f32r, name=f"wv{e}") for e in range(NE)]
                pm_t = [xw.tile([128, 1], f32, name=f"pm{t}") for t in range(NLT)]
                for e in range(NE):
                    nc.sync.dma_start(xt_t[e][:, :], xT[e * 128:(e + 1) * 128, :])
                    nc.sync.dma_start(wq_t[e][:, :], wqT[e * 128:(e + 1) * 128, :])
                    nc.sync.dma_start(wk_t[e][:, :], wkT[e * 128:(e + 1) * 128, :])
                    nc.sync.dma_start(wv_t[e][:, :], wvT[e * 128:(e + 1) * 128, :])
                for t in range(NLT):
                    nc.sync.dma_start(pm_t[t][:, :], pmask[t * 128:(t + 1) * 128, :])

                # Q^T, K^T per head: [64, L] = sum_e W^T[e, dh].T x^T[e, l]
                for h in range(HPC):
                    hs = h * Hd
                    for q in range(NQ):
                        pq = psA.tile([Hd, 512], f32, name="pq", tag="pa")
                        for e in range(NE):
                            nc.tensor.matmul(
                                pq[:, :], wq_t[e][:, hs:hs + Hd],
                                xt_t[e][:, q * 512:(q + 1) * 512],
                                start=(e == 0), stop=(e == NE - 1))
                        nc.vector.tensor_scalar_add(
                            qt_h[h][:, q * 512:(q + 1) * 512], pq[:, :],
                            bq_h[h][:, :])
                        pk = psA.tile([Hd, 512], f32, name="pk", tag="pa")
                        for e in range(NE):
                            nc.tensor.matmul(
                                pk[:, :], wk_t[e][:, hs:hs + Hd],
                                xt_t[e][:, q * 512:(q + 1) * 512],
                                start=(e == 0), stop=(e == NE - 1))
                        nc.vector.tensor_scalar_add(
                            kt_h[h][:, q * 512:(q + 1) * 512], pk[:, :],
                            bk_h[h][:, :])

                # V (natural layout) + ones column + key-padding zeroing
                for t in range(NLT):
                    pv = psA.tile([128, DHC], f32, name="pv", tag="pa")
                    for e in range(NE):
                        nc.tensor.matmul(
                            pv[:, :], xt_t[e][:, t * 128:(t + 1) * 128],
                            wv_t[e][:, :],
                            start=(e == 0), stop=(e == NE - 1))
                    # strided copy: head h -> cols 65h..65h+64
                    dst = bass.AP(vaug[t].tensor, 0,
                                  [[65 * HPC, 128], [65, HPC], [1, Hd]])
                    src = bass.AP(pv.tensor, 0,
                                  [[DHC, 128], [Hd, HPC], [1, Hd]])
                    nc.vector.tensor_copy(dst, src)
                    ones_dst = bass.AP(vaug[t].tensor, Hd,
                                       [[65 * HPC, 128], [65, HPC]])
                    nc.vector.tensor_copy(ones_dst, ones_st[:, :])
                    # zero out masked keys (numerator and denominator at once)
                    nc.vector.tensor_scalar_mul(
                        vaug[t][:, :], vaug[t][:, :], pm_t[t][:, :])

            # ---- Phase B: attention + output projection, per q-chunk ----
            with tc.tile_pool(name="eap", bufs=3) as eap, \
                 tc.tile_pool(name="wrk", bufs=3) as wrk, \
                 tc.tile_pool(name="psS", bufs=2, space="PSUM") as psS, \
                 tc.tile_pool(name="psV", bufs=HPC, space="PSUM") as psV:
                for qc in range(NQ):
                    q0 = qc * 512
                    nkt = (qc + 1) * (NLT // NQ)
                    pvh = [psV.tile([65, 512], f32, name="pvh", tag="pvh")
                           for _ in range(HPC)]
                    for kt in range(nkt):
                        ea = [None] * n_g
                        for r in range(n_g):
                            ea[r] = eap.tile([128, 512], f32, name=f"ea{r}",
                                             tag=f"ea{r}")
                            nc.sync.dma_start(
                                ea[r][:, :],
                                bass.AP(g, r * GLEN + GOFF + kt * 128 - q0,
                                        [[1, 128], [-1, 512]]))
                        for pr in range(HPC // 2):
                            h0, h1 = 2 * pr, 2 * pr + 1
                            ps2 = psS.tile([128, 1024], f32, name="ps2", tag="s")
                            nc.tensor.matmul(
                                ps2[:, 0:512],
                                kt_h[h0][:, kt * 128:(kt + 1) * 128],
                                qt_h[h0][:, q0:q0 + 512],
                                start=True, stop=True)
                            nc.tensor.matmul(
                                ps2[:, 512:1024],
                                kt_h[h1][:, kt * 128:(kt + 1) * 128],
                                qt_h[h1][:, q0:q0 + 512],
                                start=True, stop=True)
                            es2 = wrk.tile([128, 1024], f32, name="es2")
                            nc.scalar.activation(es2[:, :], ps2[:, :], Exp)
                            p2 = wrk.tile([128, 1024], f32r, name="p2")
                            nc.vector.tensor_mul(
                                p2[:, 0:512], es2[:, 0:512],
                                ea[h0 % n_g][:, :])
                            nc.vector.tensor_mul(
                                p2[:, 512:1024], es2[:, 512:1024],
                                ea[h1 % n_g][:, :])
                            nc.tensor.matmul(
                                pvh[h0][:, :], vaug[kt][:, 65 * h0:65 * h0 + 65],
                                p2[:, 0:512],
                                start=(kt == 0), stop=(kt == nkt - 1))
                            nc.tensor.matmul(
                                pvh[h1][:, :], vaug[kt][:, 65 * h1:65 * h1 + 65],
                                p2[:, 512:1024],
                                start=(kt == 0), stop=(kt == nkt - 1))

                    for h in range(HPC):
                        rec = wrk.tile([1, 512], f32r, name="rec")
                        with nc.allow_low_precision(
                                reason="softmax denom reciprocal feeds PE broadcast"):
                            nc.vector.reciprocal(rec[:, :], pvh[h][64:65, :])
                        pbc = psS.tile([Hd, 512], f32, name="pbc", tag="s")
                        nc.tensor.matmul(pbc[:, :], ones64[:, :], rec[:, :],
                                         start=True, stop=True)
                        bc = wrk.tile([Hd, 512], f32, name="bc")
                        nc.vector.tensor_copy(bc[:, :], pbc[:, :])
                        nc.vector.tensor_mul(
                            outT_h[h][:, :], pvh[h][0:64, :], bc[:, :])

                    for m in range(4):
                        for n in range(2):
                            pf = psS.tile([128, 512], f32, name="pf", tag="s")
                            for h in range(HPC):
                                nc.tensor.matmul(
                                    pf[:, :],
                                    outT_h[h][:, m * 128:(m + 1) * 128],
                                    woT_h[h][:, n * 512:(n + 1) * 512],
                                    start=(h == 0), stop=(h == HPC - 1))
                            fo = wrk.tile([128, 512], f32, name="fo")
                            nc.vector.tensor_copy(fo[:, :], pf[:, :])
                            nc.sync.dma_start(
                                out[q0 + m * 128:q0 + (m + 1) * 128,
                                    n * 512:(n + 1) * 512],
                                fo[:, :])

    _split_multi_waits(nc)
    return nc


_NC_CACHE = {}
_last_in_maps = None
_last_n_g = 1


def _get_nc(n_g):
    if n_g not in _NC_CACHE:
        _NC_CACHE[n_g] = build_nc(n_g)
    return _NC_CACHE[n_g]


def kernel(x, causal_mask, key_padding_mask, Wq, bq, Wk, bk, Wv, bv, Wo, bo,
           decay_logit):
    x = np.asarray(x, dtype=np.float32)
    Wq = np.asarray(Wq, dtype=np.float32)
    Wk = np.asarray(Wk, dtype=np.float32)
    Wv = np.asarray(Wv, dtype=np.float32)
    Wo = np.asarray(Wo, dtype=np.float32)
    bq = np.asarray(bq, dtype=np.float32)
    bk = np.asarray(bk, dtype=np.float32)
    bv = np.asarray(bv, dtype=np.float32)
    bo = np.asarray(bo, dtype=np.float32)
    decay_logit = np.asarray(decay_logit, dtype=np.float32)
    key_padding_mask = np.asarray(key_padding_mask)

    scale = 1.0 / math.sqrt(Hd)
    rates = np.log1p(np.exp(decay_logit.astype(np.float64)))  # softplus [H]

    def g_vec(rate):
        d = np.arange(GLEN) - GOFF           # d = q - k in [-2047, 2047]
        vals = np.where(d >= 0, (1.0 + np.abs(d)) ** (-rate), 0.0)
        # device AP reads g[GOFF + k - q] => store reversed
        return vals[::-1].astype(np.float32)

    in_maps = []
    n_g_needed = 1
    for c in range(N_CORES):
        b = c // CPB
        hs = (c % CPB) * HPC                 # first head of this core
        sl = slice(hs * Hd, (hs + HPC) * Hd)
        core_rates = rates[hs:hs + HPC]
        if not np.allclose(core_rates, core_rates[0], rtol=1e-6, atol=1e-9):
            n_g_needed = HPC
        gmat = (np.stack([g_vec(core_rates[0])])
                if n_g_needed == 1
                else np.stack([g_vec(r) for r in core_rates]))
        in_maps.append({
            "xT": np.ascontiguousarray(x[b].T),
            "wqT": np.ascontiguousarray((Wq[sl] * scale).T),
            "wkT": np.ascontiguousarray(Wk[sl].T),
            "wvT": np.ascontiguousarray(Wv[sl].T),
            "woT": np.ascontiguousarray(Wo[:, sl].T),
            "bq": np.ascontiguousarray((bq[sl] * scale).reshape(DHC, 1)),
            "bk": np.ascontiguousarray(bk[sl].reshape(DHC, 1)),
            "g": gmat,
            "pmask": np.ascontiguousarray(
                (~key_padding_mask[b]).astype(np.float32).reshape(L, 1)),
        })

    global _last_in_maps, _last_n_g
    _last_in_maps, _last_n_g = in_maps, n_g_needed
    nc = _get_nc(n_g_needed)
    res = bass_utils.run_bass_kernel_spmd(
        nc, in_maps, core_ids=list(range(N_CORES)))

    # q-independent constant: Wo @ bv + bo (softmax rows sum to 1)
    const = Wo.astype(np.float64) @ bv.astype(np.float64) + bo
    out = np.zeros((B, L, D), dtype=np.float64)
    for c in range(N_CORES):
        out[c // CPB] += res.results[c]["out"]
    out += const[None, None, :]
    return out.astype(np.float32)


# revision 8
# speedup vs baseline: 1.0715x; 1.0715x over previous
"""DecayAttention Trainium2 kernel — 8-core SPMD.

Problem: B=2, L=2048, D=1024, H=16 heads (Hd=64).
  out = (softmax(Q K^T/sqrt(Hd) - rate_h*log1p(|i-j|) + causal) V) @ Wo.T + bo

Sharding: core c handles batch b = c//4 and heads h in [4*(c%4), 4*(c%4)+4).
Q/K/V projections column-sharded, Wo row-sharded; the 4 cores of each batch
return partial outputs that the host sums (plus Wo@bv + bo, both of which are
q-independent constants because softmax rows sum to 1).

Device-side layout tricks:
  - x is shipped pre-transposed (xT [D, L]) so every matmul contraction dim
    sits on partitions; no on-device transposes at all.
  - Q^T/K^T [64, L] per head come straight out of the projection matmuls.
  - scores are computed transposed (S^T[k, q] = K Q^T) so softmax's k-reduction
    becomes a matmul contraction: V is augmented with a ones column and
    P^T = exp(S^T) * expA gives numerator and denominator in one PV matmul.
  - decay bias + causal mask collapse into one Toeplitz factor
    expA[k, q] = (1+|q-k|)^(-rate) * [k <= q], materialized per tile by a
    single DMA from a 4095-float vector with a diagonal access pattern
    [[1, 128], [-1, 512]].
  - matmul operands use float32r (TF32, full PE rate at N>=256).
"""
import math

import numpy as np

import concourse.bass as bass
import concourse.mybir as mybir
import concourse.tile as tile
from concourse import bass_utils

f32 = mybir.dt.float32
f32r = mybir.dt.float32r
Exp = mybir.ActivationFunctionType.Exp

B, L, D, H = 2, 2048, 1024, 16
Hd = D // H                      # 64
N_CORES = 8
CPB = N_CORES // B               # 4 cores per batch element
HPC = H // CPB                   # 4 heads per core
DHC = HPC * Hd                   # 256 head-dims per core
NQ = L // 512                    # 4 q-chunks of 512
NLT = L // 128                   # 16 l/k tiles of 128
NE = D // 128                    # 8 contraction tiles for projections
GLEN = 2 * L - 1                 # 4095
GOFF = L - 1                     # 2047


def _split_multi_waits(nc):
    """This container's walrus accepts at most one sync-wait per engine
    instruction; hoist extras onto single-wait NOPs placed just before."""
    for fn in nc.m.functions:
        for blk in fn.blocks:
            out, changed = [], False
            for inst in blk.instructions:
                si = inst.sync_info
                waits = list(si.on_wait) if si is not None and si.on_wait else []
                if len(waits) > 1:
                    changed = True
                    for w in waits[:-1]:
                        nop = mybir.InstNoOp(
                            name=nc.get_next_instruction_name(), ins=[], outs=[])
                        nop.engine = inst.engine
                        nop.sync_info = mybir.SyncInfo(on_wait=[w], on_update=[])
                        out.append(nop)
                    inst.sync_info = mybir.SyncInfo(
                        on_wait=[waits[-1]], on_update=list(si.on_update or []))
                out.append(inst)
            if changed:
                blk.instructions = out


def build_nc(n_g: int, phases=("A", "B", "WO")):
    """Build the per-core Bass program. n_g = 1 (all heads share one decay
    rate, the setup_inputs case) or HPC (per-head expA vectors)."""
    nc = bass.Bass("TRN2", target_bir_lowering=False, debug=False)

    xT = nc.dram_tensor("xT", [D, L], f32r, kind="ExternalInput").ap()
    wqT = nc.dram_tensor("wqT", [D, DHC], f32r, kind="ExternalInput").ap()
    wkT = nc.dram_tensor("wkT", [D, DHC], f32r, kind="ExternalInput").ap()
    wvT = nc.dram_tensor("wvT", [D, DHC], f32r, kind="ExternalInput").ap()
    woT = nc.dram_tensor("woT", [DHC, D], f32r, kind="ExternalInput").ap()
    bq = nc.dram_tensor("bq", [DHC, 1], f32, kind="ExternalInput").ap()
    bk = nc.dram_tensor("bk", [DHC, 1], f32, kind="ExternalInput").ap()
    g = nc.dram_tensor("g", [n_g, GLEN], f32, kind="ExternalInput")
    pmask = nc.dram_tensor("pmask", [L, 1], f32, kind="ExternalInput").ap()
    out = nc.dram_tensor("out", [L, D], f32, kind="ExternalOutput").ap()

    with tile.TileContext(nc) as tc:
        with tc.tile_pool(name="cons", bufs=1) as cons:
            # persistent SBUF residents
            qt_h = [cons.tile([Hd, L], f32r, name=f"qt{h}") for h in range(HPC)]
            kt_h = [cons.tile([Hd, L], f32r, name=f"kt{h}") for h in range(HPC)]
            vaug = [cons.tile([128, 65 * HPC], f32r, name=f"vaug{t}")
                    for t in range(NLT)]
            woT_h = [cons.tile([Hd, D], f32r, name=f"wo{h}") for h in range(HPC)]
            bq_h = [cons.tile([Hd, 1], f32, name=f"bq{h}") for h in range(HPC)]
            bk_h = [cons.tile([Hd, 1], f32, name=f"bk{h}") for h in range(HPC)]
            ones64 = cons.tile([1, Hd], f32r)
            ones_st = cons.tile([128, HPC], f32)
            outT_h = [cons.tile([Hd, 512], f32r, name=f"ot{h}") for h in range(HPC)]

            ones_st64 = cons.tile([1, Hd], f32)
            nc.vector.memset(ones_st[:, :], 1.0)
            nc.vector.memset(ones_st64[:, :], 1.0)
            nc.vector.tensor_copy(ones64[:, :], ones_st64[:, :])
            for h in range(HPC):
                nc.sync.dma_start(woT_h[h][:, :], woT[h * Hd:(h + 1) * Hd, :])
                nc.sync.dma_start(bq_h[h][:, :], bq[h * Hd:(h + 1) * Hd, :])
                nc.sync.dma_start(bk_h[h][:, :], bk[h * Hd:(h + 1) * Hd, :])

            # ---- Phase A: projections ----
            with tc.tile_pool(name="xw", bufs=1) as xw, \
                 tc.tile_pool(name="psA", bufs=3, space="PSUM") as psA, \
                 tc.tile_pool(name="wkA", bufs=3) as wkA:
                xt_t = [xw.tile([128, L], f32r, name=f"x{e}") for e in range(NE)]
                wq_t = [xw.tile([128, DHC], f32r, name=f"wq{e}") for e in range(NE)]
                wk_t = [xw.tile([128, DHC], f32r, name=f"wk{e}") for e in range(NE)]
                wv_t = [xw.tile([128, DHC], f32r, name=f"wv{e}") for e in range(NE)]
                pm_t = [xw.tile([128, 1], f32, name=f"pm{t}") for t in range(NLT)]
                for e in range(NE):
                    nc.sync.dma_start(xt_t[e][:, :], xT[e * 128:(e + 1) * 128, :])
                    nc.sync.dma_start(wq_t[e][:, :], wqT[e * 128:(e + 1) * 128, :])
                    nc.sync.dma_start(wk_t[e][:, :], wkT[e * 128:(e + 1) * 128, :])
                    nc.sync.dma_start(wv_t[e][:, :], wvT[e * 128:(e + 1) * 128, :])
                for t in range(NLT):
                    nc.sync.dma_start(pm_t[t][:, :], pmask[t * 128:(t + 1) * 128, :])

                # Q^T, K^T per head: [64, L] = sum_e W^T[e, dh].T x^T[e, l]
                for h in range(HPC):
                    hs = h * Hd
                    for q in range(NQ):
                        pq = psA.tile([Hd, 512], f32, name="pq", tag="pa")
                        for e in range(NE):
                            nc.tensor.matmul(
                                pq[:, :], wq_t[e][:, hs:hs + Hd],
                                xt_t[e][:, q * 512:(q + 1) * 512],
                                start=(e == 0), stop=(e == NE - 1))
                        nc.vector.tensor_scalar_add(
                            qt_h[h][:, q * 512:(q + 1) * 512], pq[:, :],
                            bq_h[h][:, :])
                        pk = psA.tile([Hd, 512], f32, name="pk", tag="pa")
                        for e in range(NE):
                            nc.tensor.matmul(
                                pk[:, :], wk_t[e][:, hs:hs + Hd],
                                xt_t[e][:, q * 512:(q + 1) * 512],
                                start=(e == 0), stop=(e == NE - 1))
                        nc.vector.tensor_scalar_add(
                            kt_h[h][:, q * 512:(q + 1) * 512], pk[:, :],
                            bk_h[h][:, :])

                # V (natural layout) + ones column + key-padding zeroing
                for t in range(NLT):
                    pv = psA.tile([128, DHC], f32, name="pv", tag="pa")
                    for e in range(NE):
                        nc.tensor.matmul(
                            pv[:, :], xt_t[e][:, t * 128:(t + 1) * 128],
                            wv_t[e][:, :],
                            start=(e == 0), stop=(e == NE - 1))
                    # strided copy: head h -> cols 65h..65h+64
                    dst = bass.AP(vaug[t].tensor, 0,
                                  [[65 * HPC, 128], [65, HPC], [1, Hd]])
                    src = bass.AP(pv.tensor, 0,
                                  [[DHC, 128], [Hd, HPC], [1, Hd]])
                    nc.vector.tensor_copy(dst, src)
                    ones_dst = bass.AP(vaug[t].tensor, Hd,
                                       [[65 * HPC, 128], [65, HPC]])
                    nc.vector.tensor_copy(ones_dst, ones_st[:, :])
                    # zero out masked keys (numerator and denominator at once)
                    nc.vector.tensor_scalar_mul(
                        vaug[t][:, :], vaug[t][:, :], pm_t[t][:, :])

            # ---- Phase B: attention + output projection, per q-chunk ----
            with tc.tile_pool(name="eap", bufs=3) as eap, \
                 tc.tile_pool(name="wrk", bufs=3) as wrk, \
                 tc.tile_pool(name="psS", bufs=2, space="PSUM") as psS, \
                 tc.tile_pool(name="psV", bufs=HPC, space="PSUM") as psV:
                for qc in range(NQ if "B" in phases else 0):
                    q0 = qc * 512
                    nkt = (qc + 1) * (NLT // NQ)
                    pvh = [psV.tile([65, 512], f32, name="pvh", tag="pvh")
                           for _ in range(HPC)]
                    for kt in range(nkt):
                        # ea tiles hold expA reversed along q so the DMA is
                        # contiguous; consumers read them with free step -1.
                        ea = [None] * n_g
                        earev = [None] * n_g
                        for r in range(n_g):
                            ea[r] = eap.tile([128, 512], f32, name=f"ea{r}",
                                             tag=f"ea{r}")
                            nc.sync.dma_start(
                                ea[r][:, :],
                                bass.AP(g, r * GLEN + GOFF + kt * 128 - q0 - 511,
                                        [[1, 128], [1, 512]]))
                            base = ea[r][:, :]
                            pitch = base.ap[0][0]
                            earev[r] = bass.AP(ea[r].tensor, base.offset + 511,
                                               [[pitch, 128], [-1, 512]])
                        for pr in range(HPC // 2):
                            h0, h1 = 2 * pr, 2 * pr + 1
                            ps2 = psS.tile([128, 1024], f32, name="ps2", tag="s")
                            nc.tensor.matmul(
                                ps2[:, 0:512],
                                kt_h[h0][:, kt * 128:(kt + 1) * 128],
                                qt_h[h0][:, q0:q0 + 512],
                                start=True, stop=True)
                            nc.tensor.matmul(
                                ps2[:, 512:1024],
                                kt_h[h1][:, kt * 128:(kt + 1) * 128],
                                qt_h[h1][:, q0:q0 + 512],
                                start=True, stop=True)
                            es2 = wrk.tile([128, 1024], f32, name="es2")
                            nc.scalar.activation(es2[:, :], ps2[:, :], Exp)
                            p2 = wrk.tile([128, 1024], f32r, name="p2")
                            nc.vector.tensor_mul(
                                p2[:, 0:512], es2[:, 0:512],
                                earev[h0 % n_g])
                            nc.vector.tensor_mul(
                                p2[:, 512:1024], es2[:, 512:1024],
                                earev[h1 % n_g])
                            nc.tensor.matmul(
                                pvh[h0][:, :], vaug[kt][:, 65 * h0:65 * h0 + 65],
                                p2[:, 0:512],
                                start=(kt == 0), stop=(kt == nkt - 1))
                            nc.tensor.matmul(
                                pvh[h1][:, :], vaug[kt][:, 65 * h1:65 * h1 + 65],
                                p2[:, 512:1024],
                                start=(kt == 0), stop=(kt == nkt - 1))

                    for h in range(HPC):
                        rec = wrk.tile([1, 512], f32r, name="rec")
                        with nc.allow_low_precision(
                                reason="softmax denom reciprocal feeds PE broadcast"):
                            nc.vector.reciprocal(rec[:, :], pvh[h][64:65, :])
                        pbc = psS.tile([Hd, 512], f32, name="pbc", tag="s")
                        nc.tensor.matmul(pbc[:, :], ones64[:, :], rec[:, :],
                                         start=True, stop=True)
                        bc = wrk.tile([Hd, 512], f32, name="bc")
                        nc.vector.tensor_copy(bc[:, :], pbc[:, :])
                        nc.vector.tensor_mul(
                            outT_h[h][:, :], pvh[h][0:64, :], bc[:, :])

                    for m in range(4 if "WO" in phases else 0):
                        for n in range(2):
                            pf = psS.tile([128, 512], f32, name="pf", tag="s")
                            for h in range(HPC):
                                nc.tensor.matmul(
                                    pf[:, :],
                                    outT_h[h][:, m * 128:(m + 1) * 128],
                                    woT_h[h][:, n * 512:(n + 1) * 512],
                                    start=(h == 0), stop=(h == HPC - 1))
                            fo = wrk.tile([128, 512], f32, name="fo")
                            nc.vector.tensor_copy(fo[:, :], pf[:, :])
                            nc.sync.dma_start(
                                out[q0 + m * 128:q0 + (m + 1) * 128,
                                    n * 512:(n + 1) * 512],
                                fo[:, :])

    _split_multi_waits(nc)
    return nc


_NC_CACHE = {}
_last_in_maps = None
_last_n_g = 1


def _get_nc(n_g):
    if n_g not in _NC_CACHE:
        _NC_CACHE[n_g] = build_nc(n_g)
    return _NC_CACHE[n_g]


def kernel(x, causal_mask, key_padding_mask, Wq, bq, Wk, bk, Wv, bv, Wo, bo,
           decay_logit):
    x = np.asarray(x, dtype=np.float32)
    Wq = np.asarray(Wq, dtype=np.float32)
    Wk = np.asarray(Wk, dtype=np.float32)
    Wv = np.asarray(Wv, dtype=np.float32)
    Wo = np.asarray(Wo, dtype=np.float32)
    bq = np.asarray(bq, dtype=np.float32)
    bk = np.asarray(bk, dtype=np.float32)
    bv = np.asarray(bv, dtype=np.float32)
    bo = np.asarray(bo, dtype=np.float32)
    decay_logit = np.asarray(decay_logit, dtype=np.float32)
    key_padding_mask = np.asarray(key_padding_mask)

    scale = 1.0 / math.sqrt(Hd)
    rates = np.log1p(np.exp(decay_logit.astype(np.float64)))  # softplus [H]

    def g_vec(rate):
        d = np.arange(GLEN) - GOFF           # d = q - k in [-2047, 2047]
        vals = np.where(d >= 0, (1.0 + np.abs(d)) ** (-rate), 0.0)
        # device AP reads g[GOFF + k - q] => store reversed
        return vals[::-1].astype(np.float32)

    in_maps = []
    n_g_needed = 1
    for c in range(N_CORES):
        b = c // CPB
        hs = (c % CPB) * HPC                 # first head of this core
        sl = slice(hs * Hd, (hs + HPC) * Hd)
        core_rates = rates[hs:hs + HPC]
        if not np.allclose(core_rates, core_rates[0], rtol=1e-6, atol=1e-9):
            n_g_needed = HPC
        gmat = (np.stack([g_vec(core_rates[0])])
                if n_g_needed == 1
                else np.stack([g_vec(r) for r in core_rates]))
        in_maps.append({
            "xT": np.ascontiguousarray(x[b].T),
            "wqT": np.ascontiguousarray((Wq[sl] * scale).T),
            "wkT": np.ascontiguousarray(Wk[sl].T),
            "wvT": np.ascontiguousarray(Wv[sl].T),
            "woT": np.ascontiguousarray(Wo[:, sl].T),
            "bq": np.ascontiguousarray((bq[sl] * scale).reshape(DHC, 1)),
            "bk": np.ascontiguousarray(bk[sl].reshape(DHC, 1)),
            "g": gmat,
            "pmask": np.ascontiguousarray(
                (~key_padding_mask[b]).astype(np.float32).reshape(L, 1)),
        })

    global _last_in_maps, _last_n_g
    _last_in_maps, _last_n_g = in_maps, n_g_needed
    nc = _get_nc(n_g_needed)
    res = bass_utils.run_bass_kernel_spmd(
        nc, in_maps, core_ids=list(range(N_CORES)))

    # q-independent constant: Wo @ bv + bo (softmax rows sum to 1)
    const = Wo.astype(np.float64) @ bv.astype(np.float64) + bo
    out = np.zeros((B, L, D), dtype=np.float64)
    for c in range(N_CORES):
        out[c // CPB] += res.results[c]["out"]
    out += const[None, None, :]
    return out.astype(np.float32)


# revision 9
# speedup vs baseline: 44.7932x; 41.8034x over previous
"""DecayAttention Trainium2 kernel — 8-core SPMD.

Problem: B=2, L=2048, D=1024, H=16 heads (Hd=64).
  out = (softmax(Q K^T/sqrt(Hd) - rate_h*log1p(|i-j|) + causal) V) @ Wo.T + bo

Sharding: core c handles batch b = c//4 and heads h in [4*(c%4), 4*(c%4)+4).
Q/K/V projections column-sharded, Wo row-sharded; the 4 cores of each batch
return partial outputs that the host sums (plus Wo@bv + bo, both of which are
q-independent constants because softmax rows sum to 1).

Device-side layout tricks:
  - x is shipped pre-transposed (xT [D, L]) so every matmul contraction dim
    sits on partitions; no on-device transposes at all.
  - Q^T/K^T [64, L] per head come straight out of the projection matmuls.
  - scores are computed transposed (S^T[k, q] = K Q^T) so softmax's k-reduction
    becomes a matmul contraction: V is augmented with a ones column and
    P^T = exp(S^T) * expA gives numerator and denominator in one PV matmul.
  - decay bias + causal mask collapse into one Toeplitz factor
    expA[k, q] = (1+|q-k|)^(-rate) * [k <= q], materialized per tile by a
    single DMA from a 4095-float vector with a diagonal access pattern
    [[1, 128], [-1, 512]].
  - matmul operands use float32r (TF32, full PE rate at N>=256).
"""
import math

import numpy as np

import concourse.bass as bass
import concourse.mybir as mybir
import concourse.tile as tile
from concourse import bass_utils

f32 = mybir.dt.float32
f32r = mybir.dt.float32r
Exp = mybir.ActivationFunctionType.Exp

B, L, D, H = 2, 2048, 1024, 16
Hd = D // H                      # 64
N_CORES = 8
CPB = N_CORES // B               # 4 cores per batch element
HPC = H // CPB                   # 4 heads per core
DHC = HPC * Hd                   # 256 head-dims per core
NQ = L // 512                    # 4 q-chunks of 512
NLT = L // 128                   # 16 l/k tiles of 128
NE = D // 128                    # 8 contraction tiles for projections
GLEN = 2 * L - 1                 # 4095
GOFF = L - 1                     # 2047


def _split_multi_waits(nc):
    """This container's walrus accepts at most one sync-wait per engine
    instruction; hoist extras onto single-wait NOPs placed just before."""
    for fn in nc.m.functions:
        for blk in fn.blocks:
            out, changed = [], False
            for inst in blk.instructions:
                si = inst.sync_info
                waits = list(si.on_wait) if si is not None and si.on_wait else []
                if len(waits) > 1:
                    changed = True
                    for w in waits[:-1]:
                        nop = mybir.InstNoOp(
                            name=nc.get_next_instruction_name(), ins=[], outs=[])
                        nop.engine = inst.engine
                        nop.sync_info = mybir.SyncInfo(on_wait=[w], on_update=[])
                        out.append(nop)
                    inst.sync_info = mybir.SyncInfo(
                        on_wait=[waits[-1]], on_update=list(si.on_update or []))
                out.append(inst)
            if changed:
                blk.instructions = out


def build_nc(n_g: int, phases=("A", "B", "WO"), repeat=1):
    """Build the per-core Bass program. n_g = 1 (all heads share one decay
    rate, the setup_inputs case) or HPC (per-head expA vectors)."""
    nc = bass.Bass("TRN2", target_bir_lowering=False, debug=False)

    xT = nc.dram_tensor("xT", [D, L], f32r, kind="ExternalInput").ap()
    wqT = nc.dram_tensor("wqT", [D, DHC], f32r, kind="ExternalInput").ap()
    wkT = nc.dram_tensor("wkT", [D, DHC], f32r, kind="ExternalInput").ap()
    wvT = nc.dram_tensor("wvT", [D, DHC], f32r, kind="ExternalInput").ap()
    woT = nc.dram_tensor("woT", [DHC, D], f32r, kind="ExternalInput").ap()
    bq = nc.dram_tensor("bq", [DHC, 1], f32, kind="ExternalInput").ap()
    bk = nc.dram_tensor("bk", [DHC, 1], f32, kind="ExternalInput").ap()
    g = nc.dram_tensor("g", [n_g, GLEN], f32, kind="ExternalInput")
    pmask = nc.dram_tensor("pmask", [L, 1], f32, kind="ExternalInput").ap()
    out = nc.dram_tensor("out", [L, D], f32, kind="ExternalOutput").ap()

    with tile.TileContext(nc) as tc:
      for _rep in range(repeat):
        with tc.tile_pool(name="cons", bufs=1) as cons:
            # persistent SBUF residents
            qt_h = [cons.tile([Hd, L], f32r, name=f"qt{h}") for h in range(HPC)]
            kt_h = [cons.tile([Hd, L], f32r, name=f"kt{h}") for h in range(HPC)]
            vaug = [cons.tile([128, 65 * HPC], f32r, name=f"vaug{t}")
                    for t in range(NLT)]
            woT_h = [cons.tile([Hd, D], f32r, name=f"wo{h}") for h in range(HPC)]
            bq_h = [cons.tile([Hd, 1], f32, name=f"bq{h}") for h in range(HPC)]
            bk_h = [cons.tile([Hd, 1], f32, name=f"bk{h}") for h in range(HPC)]
            ones64 = cons.tile([1, Hd], f32r)
            ones_st = cons.tile([128, HPC], f32)
            outT_h = [cons.tile([Hd, 512], f32r, name=f"ot{h}") for h in range(HPC)]

            ones_st64 = cons.tile([1, Hd], f32)
            nc.vector.memset(ones_st[:, :], 1.0)
            nc.vector.memset(ones_st64[:, :], 1.0)
            nc.vector.tensor_copy(ones64[:, :], ones_st64[:, :])
            for h in range(HPC):
                nc.sync.dma_start(woT_h[h][:, :], woT[h * Hd:(h + 1) * Hd, :])
                nc.sync.dma_start(bq_h[h][:, :], bq[h * Hd:(h + 1) * Hd, :])
                nc.sync.dma_start(bk_h[h][:, :], bk[h * Hd:(h + 1) * Hd, :])

            # ---- Phase A: projections ----
            with tc.tile_pool(name="xw", bufs=1) as xw, \
                 tc.tile_pool(name="psA", bufs=3, space="PSUM") as psA, \
                 tc.tile_pool(name="wkA", bufs=3) as wkA:
                xt_t = [xw.tile([128, L], f32r, name=f"x{e}") for e in range(NE)]
                wq_t = [xw.tile([128, DHC], f32r, name=f"wq{e}") for e in range(NE)]
                wk_t = [xw.tile([128, DHC], f32r, name=f"wk{e}") for e in range(NE)]
                wv_t = [xw.tile([128, DHC], f32r, name=f"wv{e}") for e in range(NE)]
                pm_t = [xw.tile([128, 1], f32, name=f"pm{t}") for t in range(NLT)]
                for e in range(NE):
                    nc.sync.dma_start(xt_t[e][:, :], xT[e * 128:(e + 1) * 128, :])
                    nc.sync.dma_start(wq_t[e][:, :], wqT[e * 128:(e + 1) * 128, :])
                    nc.sync.dma_start(wk_t[e][:, :], wkT[e * 128:(e + 1) * 128, :])
                    nc.sync.dma_start(wv_t[e][:, :], wvT[e * 128:(e + 1) * 128, :])
                for t in range(NLT):
                    nc.sync.dma_start(pm_t[t][:, :], pmask[t * 128:(t + 1) * 128, :])

                # Q^T, K^T per head: [64, L] = sum_e W^T[e, dh].T x^T[e, l]
                for h in range(HPC):
                    hs = h * Hd
                    for q in range(NQ):
                        pq = psA.tile([Hd, 512], f32, name="pq", tag="pa")
                        for e in range(NE):
                            nc.tensor.matmul(
                                pq[:, :], wq_t[e][:, hs:hs + Hd],
                                xt_t[e][:, q * 512:(q + 1) * 512],
                                start=(e == 0), stop=(e == NE - 1))
                        nc.vector.tensor_scalar_add(
                            qt_h[h][:, q * 512:(q + 1) * 512], pq[:, :],
                            bq_h[h][:, :])
                        pk = psA.tile([Hd, 512], f32, name="pk", tag="pa")
                        for e in range(NE):
                            nc.tensor.matmul(
                                pk[:, :], wk_t[e][:, hs:hs + Hd],
                                xt_t[e][:, q * 512:(q + 1) * 512],
                                start=(e == 0), stop=(e == NE - 1))
                        nc.vector.tensor_scalar_add(
                            kt_h[h][:, q * 512:(q + 1) * 512], pk[:, :],
                            bk_h[h][:, :])

                # V (natural layout) + ones column + key-padding zeroing
                for t in range(NLT):
                    pv = psA.tile([128, DHC], f32, name="pv", tag="pa")
                    for e in range(NE):
                        nc.tensor.matmul(
                            pv[:, :], xt_t[e][:, t * 128:(t + 1) * 128],
                            wv_t[e][:, :],
                            start=(e == 0), stop=(e == NE - 1))
                    # strided copy: head h -> cols 65h..65h+64
                    dst = bass.AP(vaug[t].tensor, 0,
                                  [[65 * HPC, 128], [65, HPC], [1, Hd]])
                    src = bass.AP(pv.tensor, 0,
                                  [[DHC, 128], [Hd, HPC], [1, Hd]])
                    nc.vector.tensor_copy(dst, src)
                    ones_dst = bass.AP(vaug[t].tensor, Hd,
                                       [[65 * HPC, 128], [65, HPC]])
                    nc.vector.tensor_copy(ones_dst, ones_st[:, :])
                    # zero out masked keys (numerator and denominator at once)
                    nc.vector.tensor_scalar_mul(
                        vaug[t][:, :], vaug[t][:, :], pm_t[t][:, :])

            # ---- Phase B: attention + output projection, per q-chunk ----
            with tc.tile_pool(name="eap", bufs=3) as eap, \
                 tc.tile_pool(name="wrk", bufs=3) as wrk, \
                 tc.tile_pool(name="psS", bufs=2, space="PSUM") as psS, \
                 tc.tile_pool(name="psV", bufs=HPC, space="PSUM") as psV:
                for qc in range(NQ if "B" in phases else 0):
                    q0 = qc * 512
                    nkt = (qc + 1) * (NLT // NQ)
                    pvh = [psV.tile([65, 512], f32, name="pvh", tag="pvh")
                           for _ in range(HPC)]
                    for kt in range(nkt):
                        # ea tiles hold expA reversed along q so the DMA is
                        # contiguous; consumers read them with free step -1.
                        ea = [None] * n_g
                        earev = [None] * n_g
                        for r in range(n_g):
                            ea[r] = eap.tile([128, 512], f32, name=f"ea{r}",
                                             tag=f"ea{r}")
                            nc.sync.dma_start(
                                ea[r][:, :],
                                bass.AP(g, r * GLEN + GOFF + kt * 128 - q0 - 511,
                                        [[1, 128], [1, 512]]))
                            base = ea[r][:, :]
                            pitch = base.ap[0][0]
                            earev[r] = bass.AP(ea[r].tensor, base.offset + 511,
                                               [[pitch, 128], [-1, 512]])
                        for pr in range(HPC // 2):
                            h0, h1 = 2 * pr, 2 * pr + 1
                            ps2 = psS.tile([128, 1024], f32, name="ps2", tag="s")
                            nc.tensor.matmul(
                                ps2[:, 0:512],
                                kt_h[h0][:, kt * 128:(kt + 1) * 128],
                                qt_h[h0][:, q0:q0 + 512],
                                start=True, stop=True)
                            nc.tensor.matmul(
                                ps2[:, 512:1024],
                                kt_h[h1][:, kt * 128:(kt + 1) * 128],
                                qt_h[h1][:, q0:q0 + 512],
                                start=True, stop=True)
                            es2 = wrk.tile([128, 1024], f32, name="es2")
                            nc.scalar.activation(es2[:, :], ps2[:, :], Exp)
                            p2 = wrk.tile([128, 1024], f32r, name="p2")
                            nc.vector.tensor_mul(
                                p2[:, 0:512], es2[:, 0:512],
                                earev[h0 % n_g])
                            nc.vector.tensor_mul(
                                p2[:, 512:1024], es2[:, 512:1024],
                                earev[h1 % n_g])
                            nc.tensor.matmul(
                                pvh[h0][:, :], vaug[kt][:, 65 * h0:65 * h0 + 65],
                                p2[:, 0:512],
                                start=(kt == 0), stop=(kt == nkt - 1))
                            nc.tensor.matmul(
                                pvh[h1][:, :], vaug[kt][:, 65 * h1:65 * h1 + 65],
                                p2[:, 512:1024],
                                start=(kt == 0), stop=(kt == nkt - 1))

                    for h in range(HPC):
                        rec = wrk.tile([1, 512], f32r, name="rec")
                        with nc.allow_low_precision(
                                reason="softmax denom reciprocal feeds PE broadcast"):
                            nc.vector.reciprocal(rec[:, :], pvh[h][64:65, :])
                        pbc = psS.tile([Hd, 512], f32, name="pbc", tag="s")
                        nc.tensor.matmul(pbc[:, :], ones64[:, :], rec[:, :],
                                         start=True, stop=True)
                        bc = wrk.tile([Hd, 512], f32, name="bc")
                        nc.vector.tensor_copy(bc[:, :], pbc[:, :])
                        nc.vector.tensor_mul(
                            outT_h[h][:, :], pvh[h][0:64, :], bc[:, :])

                    for m in range(4 if "WO" in phases else 0):
                        for n in range(2):
                            pf = psS.tile([128, 512], f32, name="pf", tag="s")
                            for h in range(HPC):
                                nc.tensor.matmul(
                                    pf[:, :],
                                    outT_h[h][:, m * 128:(m + 1) * 128],
                                    woT_h[h][:, n * 512:(n + 1) * 512],
                                    start=(h == 0), stop=(h == HPC - 1))
                            fo = wrk.tile([128, 512], f32, name="fo")
                            nc.vector.tensor_copy(fo[:, :], pf[:, :])
                            nc.sync.dma_start(
                                out[q0 + m * 128:q0 + (m + 1) * 128,
                                    n * 512:(n + 1) * 512],
                                fo[:, :])

    _split_multi_waits(nc)
    return nc


_NC_CACHE = {}
_last_in_maps = None
_last_n_g = 1


def _get_nc(n_g):
    if n_g not in _NC_CACHE:
        _NC_CACHE[n_g] = build_nc(n_g)
    return _NC_CACHE[n_g]


def kernel(x, causal_mask, key_padding_mask, Wq, bq, Wk, bk, Wv, bv, Wo, bo,
           decay_logit):
    x = np.asarray(x, dtype=np.float32)
    Wq = np.asarray(Wq, dtype=np.float32)
    Wk = np.asarray(Wk, dtype=np.float32)
    Wv = np.asarray(Wv, dtype=np.float32)
    Wo = np.asarray(Wo, dtype=np.float32)
    bq = np.asarray(bq, dtype=np.float32)
    bk = np.asarray(bk, dtype=np.float32)
    bv = np.asarray(bv, dtype=np.float32)
    bo = np.asarray(bo, dtype=np.float32)
    decay_logit = np.asarray(decay_logit, dtype=np.float32)
    key_padding_mask = np.asarray(key_padding_mask)

    scale = 1.0 / math.sqrt(Hd)
    rates = np.log1p(np.exp(decay_logit.astype(np.float64)))  # softplus [H]

    def g_vec(rate):
        d = np.arange(GLEN) - GOFF           # d = q - k in [-2047, 2047]
        vals = np.where(d >= 0, (1.0 + np.abs(d)) ** (-rate), 0.0)
        # device AP reads g[GOFF + k - q] => store reversed
        return vals[::-1].astype(np.float32)

    in_maps = []
    n_g_needed = 1
    for c in range(N_CORES):
        b = c // CPB
        hs = (c % CPB) * HPC                 # first head of this core
        sl = slice(hs * Hd, (hs + HPC) * Hd)
        core_rates = rates[hs:hs + HPC]
        if not np.allclose(core_rates, core_rates[0], rtol=1e-6, atol=1e-9):
            n_g_needed = HPC
        gmat = (np.stack([g_vec(core_rates[0])])
                if n_g_needed == 1
                else np.stack([g_vec(r) for r in core_rates]))
        in_maps.append({
            "xT": np.ascontiguousarray(x[b].T),
            "wqT": np.ascontiguousarray((Wq[sl] * scale).T),
            "wkT": np.ascontiguousarray(Wk[sl].T),
            "wvT": np.ascontiguousarray(Wv[sl].T),
            "woT": np.ascontiguousarray(Wo[:, sl].T),
            "bq": np.ascontiguousarray((bq[sl] * scale).reshape(DHC, 1)),
            "bk": np.ascontiguousarray(bk[sl].reshape(DHC, 1)),
            "g": gmat,
            "pmask": np.ascontiguousarray(
                (~key_padding_mask[b]).astype(np.float32).reshape(L, 1)),
        })

    global _last_in_maps, _last_n_g
    _last_in_maps, _last_n_g = in_maps, n_g_needed
    nc = _get_nc(n_g_needed)
    res = bass_utils.run_bass_kernel_spmd(
        nc, in_maps, core_ids=list(range(N_CORES)))

    # q-independent constant: Wo @ bv + bo (softmax rows sum to 1)
    const = Wo.astype(np.float64) @ bv.astype(np.float64) + bo
    out = np.zeros((B, L, D), dtype=np.float64)
    for c in range(N_CORES):
        out[c // CPB] += res.results[c]["out"]
    out += const[None, None, :]
    return out.astype(np.float32)
